# revision 43
# baseline (speedup 1.0000x reference)
"""Distributed multi-head attention kernel for 8 Trainium2 NeuronCores.

Problem: y = softmax((x Wq^T)(x Wk^T)^T / sqrt(D)) (x Wv^T) Wo^T + bo
with B=4, T=2048, C=1280, H=20, D=64, float32 I/O.

Sharding (sequence parallel, rank independent):
  Each core owns a T/8 token slice of all 4 batches (1024 tokens).
  It computes Q/K/V projections for its tokens, AllGathers K^T and V
  per batch (4 pipelined AllGathers so comm overlaps compute), runs
  full attention for its queries over the gathered keys/values, and
  applies the output projection for its tokens. The host reassembles
  the T axis.

The device is power/utilization-throttled under sustained PE load, so
the design minimizes PE busy-cycles and keeps engine queues unblocked:
  - Scores run in transposed S_T[k, q] layout, two heads row-packed
    (tile_position (0,0)/(64,0)). Each exp chunk holds h0 for kt+0/1
    and h1 for kt+2/3 (mirrored in the second chunk) so paired score
    matmuls use different row groups AND different PSUM banks and
    become ready simultaneously - they launch ~4ns apart.
  - P@V runs column-tiled: both heads of a pair concurrently at M=64
    (tile_position (0,0)/(0,64)) - 2x over a padded-V M=65 scheme.
  - Softmax denominators come from 4-way column-tiled ones-matmuls
    (M=32 strips: two kt-parity streams x two heads per slot).
  - Normalization: one expander matmul broadcasts summed strips to all
    128 rows, reciprocal_approx_fast + one tensor_mul per head pair.
  - Gather-side loads (K panels, V blocks) issue on the scalar HWDGE
    queue so they are not serialized behind phase-1 bounce stores.
  - AllGathers 2 and 3 are gated behind the previous batch's V reads
    (pad-write dependency) so the serialized CC queue does not starve
    the current batch's attention loads of fabric bandwidth.
  - No filler matmuls; O-projection is interleaved into the next
    batch's attention loop to keep ScalarE (exp) saturated.

Compute dtype is bf16 (fp32 matmul is 4x slower on the PE array);
accumulation is fp32 in PSUM. I/O stays fp32.
"""

import os
import sys
import types

import numpy as np
import ml_dtypes

import concourse.bass as bass
import concourse.mybir as mybir
import concourse.tile as tile
from concourse import bacc
from concourse.bass_utils import run_bass_kernel_spmd

N_CORES = 8
C = 1280
H = 20
D = 64
B = 4
CT = C // 128  # 10 c-tiles
BF = mybir.dt.bfloat16
F32 = mybir.dt.float32
SCALE = 1.0 / (D ** 0.5)

LAST_EXEC_TIME_NS = None
_BUILD_CACHE = {}


def _install_ntff_hook():
    """The trimmed antenv package lacks axon_hooks; register the NTFF
    profile hook by hand so trace=True can time the NEFF on silicon.
    Safe no-op if anything is missing."""
    if "antenv.axon_hooks" in sys.modules:
        return
    try:
        from trn_agent_boot.trn_boot import _ntff_profile_via_ctypes

        hook = _ntff_profile_via_ctypes("/opt/axon/libaxon_pjrt.so")
        mod = types.ModuleType("antenv.axon_hooks")
        mod.get_axon_ntff_profile_hook = lambda: hook
        mod.set_axon_ntff_profile_hook = lambda h: None
        sys.modules["antenv.axon_hooks"] = mod
        import antenv

        antenv.axon_hooks = mod
    except Exception:
        pass


def _chunks(total, step):
    out = []
    o = 0
    while o < total:
        out.append((o, min(step, total - o)))
        o += step
    return out


def build(T):
    """Build the SPMD Bass graph for full (unsharded) sequence length T."""
    TS = T // N_CORES          # tokens per batch per core (256)
    TOK = B * TS               # tokens per core (1024)
    KT = T // 128              # 128-wide key tiles per batch (16)
    JR = TS // 128             # key tiles per rank per batch (2)
    assert TS % 128 == 0 and KT % 4 == 0
    C4 = KT // 4               # exp chunks (4 key tiles each) per head
    RG = [list(range(N_CORES))]
    SZK = C * TS               # K^T payload elems per batch
    SZV = TS * C               # V payload elems per batch
    SZ = SZK + SZV
    SZP = SZ + 64              # + pad gate (delays the AG start until the
                               #   previous batch's gathered reads finish)

    nc = bacc.Bacc("TRN2", target_bir_lowering=False, debug=False,
                   num_devices=N_CORES)

    xT = nc.dram_tensor("xT", [C, TOK], BF, kind="ExternalInput").ap()
    wqT = nc.dram_tensor("wqT", [C, C], BF, kind="ExternalInput").ap()
    wkT = nc.dram_tensor("wkT", [C, C], BF, kind="ExternalInput").ap()
    wvT = nc.dram_tensor("wvT", [C, C], BF, kind="ExternalInput").ap()
    woT = nc.dram_tensor("woT", [C, C], BF, kind="ExternalInput").ap()
    bo_d = nc.dram_tensor("bo", [C, 1], F32, kind="ExternalInput").ap()
    # expander: bc_den[p, q] = sum of the two den strips of head(p)
    esel_d = nc.dram_tensor("esel", [128, 128], BF, kind="ExternalInput").ap()
    out = nc.dram_tensor("out", [C, TOK], F32, kind="ExternalOutput").ap()

    with tile.TileContext(nc) as tc:
        with tc.tile_pool(name="dram", bufs=1, space="DRAM") as dram:
            kv_bn = [dram.tile([SZP], BF, name=f"kv_bn{b}") for b in range(B)]
            kv_all = [dram.tile([N_CORES * SZP], BF, addr_space="Shared",
                                name=f"kv_all{b}") for b in range(B)]
            warm_bn = dram.tile([128], BF, name="warm_bn")
            warm_all = dram.tile([N_CORES * 128], BF, addr_space="Shared",
                                 name="warm_all")

            with tc.tile_pool(name="persist", bufs=1) as persist:
                # tiny AllGather first: absorbs the CC bootstrap barrier and
                # queue warmup so the first real gather starts promptly
                warm_sb = persist.tile([1, 128], BF)
                nc.vector.memset(warm_sb[:], 0.0)
                nc.sync.dma_start(warm_bn[:].rearrange("(p f) -> p f", p=1),
                                  warm_sb[:])
                nc.gpsimd.collective_compute(
                    "AllGather", mybir.AluOpType.bypass,
                    replica_groups=RG,
                    ins=[warm_bn[:].opt()],
                    outs=[warm_all[:].opt()])

                qT_sb = persist.tile([128, CT, TOK], BF)
                attn_sb = persist.tile([128, CT, TOK], BF)
                wo_sb = persist.tile([128, CT, C], BF)
                bo_sb = persist.tile([128, CT, 1], F32)
                ones32 = persist.tile([128, 32], BF)
                nc.vector.memset(ones32[:], 1.0)
                esel_sb = persist.tile([128, 128], BF)
                nc.sync.dma_start(esel_sb[:], esel_d)

                # ---------------- Phase 1: projections + AGs ----------
                with tc.tile_pool(name="p1", bufs=1) as p1, \
                     tc.tile_pool(name="psum1", bufs=1, space="PSUM") as psum1:
                    # split loads per c-tile so the first projection chains
                    # can start while the rest still streams in
                    xT_sb = p1.tile([128, CT, TOK], BF)
                    wk_sb = p1.tile([128, CT, C], BF)
                    wv_sb = p1.tile([128, CT, C], BF)
                    wq_sb = p1.tile([128, CT, C], BF)
                    xT_v = xT.rearrange("(n p) t -> p n t", p=128)
                    wkT_v = wkT.rearrange("(n p) o -> p n o", p=128)
                    wvT_v = wvT.rearrange("(n p) o -> p n o", p=128)
                    wqT_v = wqT.rearrange("(n p) o -> p n o", p=128)
                    for i in range(CT):
                        nc.sync.dma_start(xT_sb[:, i, :], xT_v[:, i, :])
                        nc.sync.dma_start(wk_sb[:, i, :], wkT_v[:, i, :])
                    for i in range(CT):
                        nc.sync.dma_start(wv_sb[:, i, :], wvT_v[:, i, :])
                    for i in range(CT):
                        nc.sync.dma_start(wq_sb[:, i, :], wqT_v[:, i, :])

                    for half in range(2):
                        t_lo = half * 2 * TS
                        # K^T projection for this half's 512 tokens
                        for ot in range(CT):
                            ps = psum1.tile([128, 2 * TS], F32, tag="mm",
                                            bufs=4, name="ps_k")
                            for i in range(CT):
                                nc.tensor.matmul(
                                    ps[:],
                                    wk_sb[:, i, ot * 128:(ot + 1) * 128],
                                    xT_sb[:, i, t_lo:t_lo + 2 * TS],
                                    start=(i == 0), stop=(i == CT - 1))
                            st = p1.tile([128, 2 * TS], BF, tag="st",
                                         bufs=4, name="st_k")
                            nc.vector.tensor_copy(st[:], ps[:])
                            for bb in range(2):
                                b = 2 * half + bb
                                kview = kv_bn[b][0:SZK].rearrange(
                                    "(r t) -> r t", t=TS)
                                nc.sync.dma_start(
                                    kview[ot * 128:(ot + 1) * 128, :],
                                    st[:, bb * TS:(bb + 1) * TS])
                        # V projection (token-major) + AG per batch
                        for bb in range(2):
                            b = 2 * half + bb
                            vview = kv_bn[b][SZK:SZ].rearrange(
                                "(p c) -> p c", c=C)
                            for ttl in range(JR):
                                tt = b * JR + ttl
                                stv = p1.tile([128, C], BF, tag="stv",
                                              bufs=3, name="stv")
                                for o0, osz in _chunks(C, 512):
                                    ps = psum1.tile([128, 512], F32, tag="mm",
                                                    bufs=4, name="ps_v")
                                    for i in range(CT):
                                        nc.tensor.matmul(
                                            ps[:, :osz],
                                            xT_sb[:, i,
                                                  tt * 128:(tt + 1) * 128],
                                            wv_sb[:, i, o0:o0 + osz],
                                            start=(i == 0),
                                            stop=(i == CT - 1))
                                    nc.vector.tensor_copy(
                                        stv[:, o0:o0 + osz], ps[:, :osz])
                                nc.sync.dma_start(
                                    vview[ttl * 128:(ttl + 1) * 128, :],
                                    stv[:])
                            if b <= 1:
                                # AG(0)/AG(1) fire as soon as their data is
                                # stored; AG(2)/AG(3) are emitted in the
                                # attention region behind pad-write gates
                                nc.gpsimd.collective_compute(
                                    "AllGather", mybir.AluOpType.bypass,
                                    replica_groups=RG,
                                    ins=[kv_bn[b][:].opt()],
                                    outs=[kv_all[b][:].opt()])

                    # all of Q^T (overlaps the AllGathers)
                    for ot in range(CT):
                        for t0, tsz in _chunks(TOK, 512):
                            ps = psum1.tile([128, 512], F32, tag="mm",
                                            bufs=4, name="ps_q")
                            for i in range(CT):
                                nc.tensor.matmul(
                                    ps[:, :tsz],
                                    wq_sb[:, i, ot * 128:(ot + 1) * 128],
                                    xT_sb[:, i, t0:t0 + tsz],
                                    start=(i == 0), stop=(i == CT - 1))
                            nc.vector.tensor_copy(
                                qT_sb[:, ot, t0:t0 + tsz], ps[:, :tsz])

                    nc.sync.dma_start(
                        wo_sb[:], woT.rearrange("(n p) o -> p n o", p=128))
                    nc.sync.dma_start(
                        bo_sb[:], bo_d.rearrange("(n p) o -> p n o", p=128))

                # ------------- Phase 2: attention + out-proj -------------
                with tc.tile_pool(name="p2", bufs=1) as p2, \
                     tc.tile_pool(name="psum2", bufs=1, space="PSUM") as psum2:

                    def load_vbs(b):
                        # gather-side loads go on the scalar (Activation)
                        # HWDGE queue: the sync queue is serialized behind
                        # phase-1 bounce stores that wait on PE progress
                        kv_s = kv_all[b][:].rearrange("(s x) -> s x",
                                                      s=N_CORES)
                        v_all_v = kv_s[:, SZK:SZ].rearrange(
                            "s (j p c) -> s j p c", p=128, c=C)
                        vbs = []
                        for kh in range(2):
                            vb = p2.tile([128, KT // 2, C], BF, tag="vb",
                                         bufs=4, name=f"vb{kh}")
                            for s0 in range(N_CORES // 2):
                                s = kh * (N_CORES // 2) + s0
                                for j in range(JR):
                                    nc.scalar.dma_start(
                                        vb[:, s0 * JR + j, :],
                                        v_all_v[s, j])
                            vbs.append(vb)
                        if 1 <= b < B - 1:
                            # AG(b+1) gate: its bounce pad is written only
                            # after this batch's V gather has been read, so
                            # the serialized CC queue does not hog fabric
                            # bandwidth while batch b's attention loads run.
                            # The collective is emitted HERE (after the pad
                            # write) so the pad is a true RAW dependency.
                            gsb = p2.tile([1, 64], BF, tag="gate", bufs=2,
                                          name="gsb")
                            nc.vector.tensor_copy(
                                gsb[:], vbs[1][0:1, KT // 2 - 1, 0:64])
                            nc.sync.dma_start(
                                kv_bn[b + 1][SZ:SZP].rearrange(
                                    "(p f) -> p f", p=1),
                                gsb[:])
                            nc.gpsimd.collective_compute(
                                "AllGather", mybir.AluOpType.bypass,
                                replica_groups=RG,
                                ins=[kv_bn[b + 1][:].opt()],
                                outs=[kv_all[b + 1][:].opt()])
                        return vbs

                    def emit_oproj(b, co, tag="misc"):
                        psy = psum2.tile([128, TS], F32, tag=tag, bufs=1,
                                         name="psy")
                        for ct in range(CT):
                            nc.tensor.matmul(
                                psy[:],
                                wo_sb[:, ct, co * 128:(co + 1) * 128],
                                attn_sb[:, ct, b * TS:(b + 1) * TS],
                                start=(ct == 0), stop=(ct == CT - 1))
                        ysb = p2.tile([128, TS], F32, tag="y", bufs=3,
                                      name="ysb")
                        nc.vector.tensor_scalar_add(
                            ysb[:], psy[:], bo_sb[:, co, :])
                        nc.sync.dma_start(
                            out[co * 128:(co + 1) * 128,
                                b * TS:(b + 1) * TS],
                            ysb[:])

                    def k_view(b):
                        kv_s = kv_all[b][:].rearrange("(s x) -> s x",
                                                      s=N_CORES)
                        return kv_s[:, 0:SZK].rearrange(
                            "s (r t) -> r s t", t=TS)

                    # first kp BEFORE the vb block: attention's first scores
                    # only need kp + qT, so don't queue it behind 10MB of
                    # V-gather DMA traffic
                    kp0 = p2.tile([128, N_CORES, TS], BF, tag="kp",
                                  bufs=3, name="kp")
                    nc.scalar.dma_start(kp0[:], k_view(0)[0:128, :, :])
                    vbs_cur = load_vbs(0)
                    kp_pre = kp0
                    for b in range(B):
                        k_all_v = k_view(b)
                        btok = slice(b * TS, (b + 1) * TS)

                        for hp in range(CT):
                            if hp == 0:
                                kp = kp_pre
                            else:
                                kp = p2.tile([128, N_CORES, TS], BF,
                                             tag="kp", bufs=3, name="kp")
                                nc.scalar.dma_start(
                                    kp[:],
                                    k_all_v[hp * 128:(hp + 1) * 128, :, :])
                            if hp == 8 and b + 1 < B:
                                # prefetch next batch's first K panel so the
                                # batch transition does not stall on DMA
                                kp_pre = p2.tile([128, N_CORES, TS], BF,
                                                 tag="kp", bufs=3, name="kp")
                                nc.scalar.dma_start(
                                    kp_pre[:], k_view(b + 1)[0:128, :, :])
                            kp_f = kp[:].rearrange("p s t -> p (s t)")

                            # P layout per c4 block of 8*TS cols:
                            #   A-part: [h0 kt+0, h0 kt+1, h1 kt+2, h1 kt+3]
                            #   B-part: [h1 kt+0, h1 kt+1, h0 kt+2, h0 kt+3]
                            # so each exp chunk's scores pair different row
                            # groups AND different psum banks with a single
                            # reader - the pairs schedule concurrently
                            P_all = p2.tile([128, 2 * KT * TS], BF, tag="P",
                                            bufs=2, name="P_all")

                            def p_slice(hslot, kt):
                                l = kt % 4
                                in_a = (hslot == 0) == (l < 2)
                                base = ((kt // 4) * 8 * TS
                                        + (0 if in_a else 4 * TS) + l * TS)
                                return P_all[:, base:base + TS]

                            pav = psum2.tile([128, TS], F32, tag="pav",
                                             bufs=2, name="pav")
                            den = psum2.tile([128, TS], F32, tag="den",
                                             bufs=1, name="den")
                            he, ho = 2 * hp, 2 * hp + 1

                            def score_mm(sp, l, hslot, kt):
                                r = hslot * 64
                                nc.tensor.matmul(
                                    sp[:, l * TS:(l + 1) * TS],
                                    kp_f[r:r + 64,
                                         kt * 128:(kt + 1) * 128],
                                    qT_sb[r:r + 64, hp, btok],
                                    start=True, stop=True,
                                    tile_position=(r, 0))

                            for c4 in range(C4):
                                k0 = c4 * 4
                                spA = psum2.tile([128, 4 * TS], F32,
                                                 tag="spA", bufs=1,
                                                 name="spA")
                                spB = psum2.tile([128, 4 * TS], F32,
                                                 tag="spB", bufs=1,
                                                 name="spB")
                                for lp in range(2):
                                    score_mm(spA, lp, 0, k0 + lp)
                                    score_mm(spA, lp + 2, 1, k0 + lp + 2)
                                for lp in range(2):
                                    score_mm(spB, lp, 1, k0 + lp)
                                    score_mm(spB, lp + 2, 0, k0 + lp + 2)
                                nc.scalar.activation(
                                    P_all[:, c4 * 8 * TS:c4 * 8 * TS
                                          + 4 * TS],
                                    spA[:],
                                    mybir.ActivationFunctionType.Exp,
                                    scale=SCALE)
                                nc.scalar.activation(
                                    P_all[:, c4 * 8 * TS + 4 * TS:
                                          (c4 + 1) * 8 * TS],
                                    spB[:],
                                    mybir.ActivationFunctionType.Exp,
                                    scale=SCALE)
                                # P@V + denominators grouped by exp source:
                                # the A-group (h0 kt+0/1, h1 kt+2/3) only
                                # needs the first exp of this chunk, so the
                                # PE starts it a full exp earlier instead of
                                # serializing everything behind exp-B.
                                def pav_mm(hs, kt, st0, sp1):
                                    vb = vbs_cur[kt // (KT // 2)]
                                    ktl = kt % (KT // 2)
                                    h = (he, ho)[hs]
                                    nc.tensor.matmul(
                                        pav[hs * 64:hs * 64 + 64, :],
                                        vb[:, ktl, h * 64:h * 64 + 64],
                                        p_slice(hs, kt),
                                        start=st0, stop=sp1,
                                        tile_position=(0, hs * 64),
                                        skip_group_check=True)

                                def den_mm(pos, hs, kt, st0, sp1):
                                    nc.tensor.matmul(
                                        den[pos:pos + 32, :],
                                        ones32[:],
                                        p_slice(hs, kt),
                                        start=st0, stop=sp1,
                                        tile_position=(0, pos),
                                        skip_group_check=True)

                                k0 = c4 * 4
                                first, last = (c4 == 0), (c4 == C4 - 1)
                                # A-sourced: pav pairs (h0 l0|h1 l2),
                                # (h0 l1|h1 l3) + den quad covering all four
                                # kt-parity strips
                                pav_mm(0, k0 + 0, first, False)
                                pav_mm(1, k0 + 2, first, False)
                                pav_mm(0, k0 + 1, False, False)
                                pav_mm(1, k0 + 3, False, False)
                                den_mm(0, 0, k0 + 0, first, False)
                                den_mm(32, 0, k0 + 1, first, False)
                                den_mm(64, 1, k0 + 2, first, False)
                                den_mm(96, 1, k0 + 3, first, False)
                                # B-sourced (h0 ends on kt+3, h1 on kt+1)
                                pav_mm(1, k0 + 0, False, False)
                                pav_mm(0, k0 + 2, False, False)
                                pav_mm(1, k0 + 1, False, last)
                                pav_mm(0, k0 + 3, False, last)
                                den_mm(64, 1, k0 + 0, False, last)
                                den_mm(96, 1, k0 + 1, False, last)
                                den_mm(0, 0, k0 + 2, False, last)
                                den_mm(32, 0, k0 + 3, False, last)

                            # normalization for this head pair
                            den_sb = p2.tile([128, TS], BF, tag="densb",
                                             bufs=2, name="den_sb")
                            nc.vector.tensor_copy(den_sb[:], den[:])
                            bcd = psum2.tile([128, TS], F32, tag="misc",
                                             bufs=1, name="bcd")
                            nc.tensor.matmul(bcd[:], esel_sb[:], den_sb[:],
                                             start=True, stop=True)
                            recf = p2.tile([128, TS], F32, tag="recf",
                                           bufs=2, name="recf")
                            nc.vector.reciprocal_approx_fast(recf[:], bcd[:])
                            recb = p2.tile([128, TS], BF, tag="recb",
                                           bufs=2, name="recb")
                            nc.vector.tensor_copy(recb[:], recf[:])
                            nc.vector.tensor_mul(
                                attn_sb[:, hp, btok], pav[:], recb[:])

                            # interleave previous batch's out-projection
                            if b > 0:
                                emit_oproj(b - 1, hp)
                            # prefetch next batch's V mid-way through
                            if hp == 5 and b + 1 < B:
                                vbs_next = load_vbs(b + 1)

                        if b + 1 < B:
                            vbs_cur = vbs_next

                    for co in range(CT):
                        emit_oproj(B - 1, co, tag=("misc", "den")[co % 2])

    nc.compile()
    return nc


def _make_esel():
    E = np.zeros((128, 128), np.float32)
    for p in range(128):
        s = (0 if p < 64 else 64) + (p % 32)
        E[s, p] = 1.0
        E[s + 32, p] = 1.0
    return E.astype(ml_dtypes.bfloat16)


def _prep_inputs(hidden_states, Wq, Wk, Wv, Wo, bo):
    T = hidden_states.shape[1]
    TS = T // N_CORES
    TOK = B * TS
    bf = ml_dtypes.bfloat16
    wqT = np.ascontiguousarray(np.asarray(Wq, np.float32).T).astype(bf)
    wkT = np.ascontiguousarray(np.asarray(Wk, np.float32).T).astype(bf)
    wvT = np.ascontiguousarray(np.asarray(Wv, np.float32).T).astype(bf)
    woT = np.ascontiguousarray(np.asarray(Wo, np.float32).T).astype(bf)
    bo_c = np.asarray(bo, np.float32).reshape(C, 1)
    x = np.asarray(hidden_states, np.float32)
    in_maps = []
    for r in range(N_CORES):
        xr = x[:, r * TS:(r + 1) * TS, :].reshape(TOK, C)
        xTr = np.ascontiguousarray(xr.T).astype(bf)
        in_maps.append({
            "xT": xTr, "wqT": wqT, "wkT": wkT, "wvT": wvT, "woT": woT,
            "bo": bo_c, "esel": _make_esel(),
        })
    return in_maps


def kernel(hidden_states, Wq, Wk, Wv, Wo, bo):
    global LAST_EXEC_TIME_NS
    _install_ntff_hook()
    Bx, T, Cx = hidden_states.shape
    assert (Bx, Cx) == (B, C)
    TS = T // N_CORES
    if T not in _BUILD_CACHE:
        _BUILD_CACHE[T] = build(T)
    nc = _BUILD_CACHE[T]
    in_maps = _prep_inputs(hidden_states, Wq, Wk, Wv, Wo, bo)
    res = run_bass_kernel_spmd(nc, in_maps, core_ids=list(range(N_CORES)))
    LAST_EXEC_TIME_NS = res.exec_time_ns
    outf = np.empty((B, T, C), np.float32)
    for r in range(N_CORES):
        yT = res.results[r]["out"]          # [C, TOK]
        yr = yT.T.reshape(B, TS, C)
        outf[:, r * TS:(r + 1) * TS, :] = yr
    return outf
